# revision 37
# baseline (speedup 1.0000x reference)
"""Trainium2 Bass kernel for nn_Def_A2C_Sample_Generator.

Computation (see reference):
  x = concat(state, payoff, noise)            (500, 504)
  h1 = lrelu(bn(adj @ (x @ w1) + b1))         (500, 32)
  h2 = lrelu(bn(adj @ (h1 @ w2) + b2))        (500, 16)
  xf = h2.reshape(8000)
  logits = xf @ actgen_w + def_cur_loc @ actgen_v          (50, 500)
  out = softmax(logits[None] + gumbel(u), axis=-1)         (1000, 50, 500)

Sharding: data-parallel over the 1000 samples, 125 per core on 8
cores. Each core computes the logits redundantly (small GCN; the 8 MB
bf16 actgen_w is streamed) and softmaxes its own 125 x 50 x 500 gumbel
block.

Softmax is factored so every ACT pass is independent of the logits
(the logits path is the serial prologue; all gumbel work front-runs
it):
  exp(logits + g) with g = -ln(-ln u) equals L * a where
  L = exp(logits) (prologue, 50x500) and a = exp(-ln(-ln u)) = -1/ln u.
Main loop, 5-r chunks in the natural (sample, r, T) layout:
  a       : 3 chunk-wide in-place ACT passes (Ln, Ln(-x), Exp(-x);
            one table set - see the act-table monkeypatch below)
  L bcast : per-r PE ones-matmul, single bf16 plane into f32 PSUM
  q, S    : DVE scalar_tensor_tensor mult with fused row-sum accum
  out     : one DVE reciprocal per chunk + per-r tensor_scalar mult
            into a bf16 chunk tile, one 625KB store per chunk
            (host upcasts to f32; bf16 rounding is ~2e-3 rel, the
            harness gate is 2e-2)

TRN2 PE runs at a ~5x-slow mid p-state unless continuously busy for
3us, and f32 matmuls cost 4 cyc/row on top -- so every prologue
matmul operand that tolerates bf16 (adjT, av, dclT, bias rows, zrow,
xw tiles) is bf16, and the per-r broadcast is one bf16 plane (~0.2%
rounding on L, well inside the error budget).

DMA queues: u loads + params on the sync HWDGE ring (NOT the scalar
ring: HWDGE descriptor generation executes on the issuing engine, and
the scalar ring would burn ~40us of ACT engine time), actgen_w stream
+ output stores on the gpsimd SWDGE path. actgen_w is host-packed
per-partition-contiguous ([16,125,2000]) so each channel load is 125
4KB-run descriptors instead of 500 1KB ones.

Logits rows are packed into 3 lanes at base partitions 0/32/64 (the
only legal matmul operand bases) via a DRAM bounce.
"""
import sys

if "/opt/trn_rl_repo" not in sys.path:
    sys.path.insert(0, "/opt/trn_rl_repo")

import numpy as np

import concourse.bacc as bacc
import concourse.bass as bass
import concourse.mybir as mybir
import concourse.tile as tile
from concourse import bass_utils

# The act-table-load pass resolves Exp -> exp_and_others (id 0) and
# Ln -> natural_log (id 5), thrashing a ~2.7us table swap at every
# Ln<->Exp transition in the main loop. natural_log_exp_and_others
# (id 6) holds BOTH. Hide exp/ln from the other sets in the map the
# chooser reads (ids keep indexing the real act_info.json, so the
# loaded tables are unchanged) so every Exp and Ln lands on set 6 and
# one load suffices.
_orig_get_act_tables = bacc.get_activation_tables


def _patched_get_act_tables(arch):
    tabs = dict(_orig_get_act_tables(arch))
    both = {mybir.ActivationFunctionType.Exp, mybir.ActivationFunctionType.Ln}
    for name, fns in tabs.items():
        if name != "natural_log_exp_and_others" and (both & fns):
            tabs[name] = fns - both
    return tabs


bacc.get_activation_tables = _patched_get_act_tables

F32 = mybir.dt.float32
BF16 = mybir.dt.bfloat16
NCORES = 8
T = 500
R = 50
NS = 1000
SP = NS // NCORES  # 125 samples per core
H1, H2 = 32, 16
FIN = 504  # 2 + 500 + 2 input features
KT = 4  # K/M tiling of the 500 dim into 4x125
NEG_SLOPE = 0.2

_CACHE = {}


def _build():
    nc = bacc.Bacc("TRN2", target_bir_lowering=False, debug=False,
                   enable_asserts=False, num_devices=NCORES)

    # ---- I/O ----
    din = {}
    # mega-packed params: per-DMA fixed costs on the HWDGE ring are
    # ~1-2us each, so ~25 small tile loads serialize into ~45us. Two
    # packed planes load in ~8us instead.
    # pbf[p, :] = adjT k-tiles (4x500) | av k-tiles (4x500) | dclT (4x50)
    din["pbf"] = nc.dram_tensor("pbf", [125, 8 * T + 4 * R], BF16,
                                kind="ExternalInput")
    # pxb[p, :] = xT k-tiles (4x500) | w1 k-tiles (4x32), bf16
    din["pxb"] = nc.dram_tensor("pxb", [126, 4 * T + 4 * H1], BF16,
                                kind="ExternalInput")
    # rows[0, :] = b1 (32) | b2 (16) | grow (500) | brow (500)
    din["rows"] = nc.dram_tensor("rows", [1, H1 + H2 + 2 * T], BF16,
                                 kind="ExternalInput")
    din["w2"] = nc.dram_tensor("w2", [H1, H2], F32, kind="ExternalInput")
    # per-core actgen_w shard: 2 of 16 channels (the kernel is HBM-byte
    # bound at ~133GB/s/core, so the 8MB replicated stream IS the
    # bottleneck; each core computes a partial z from its 2 channels
    # and a 2KB ncfw AllReduce -- whose ~tens-of-us latency is fully
    # hidden, DVE only needs logits at ~110us -- completes it).
    # Host-side, gc2's output channels are permuted per core so the
    # owned channels sit at h2[:, 0:2]; channels only feed z.
    din["wr"] = nc.dram_tensor("wr", [2, 125, KT * T], BF16,
                               kind="ExternalInput")
    din["ident"] = nc.dram_tensor("ident", [128, 128], F32, kind="ExternalInput")
    din["u"] = nc.dram_tensor("u", [SP, R, T], F32, kind="ExternalInput")
    out = nc.dram_tensor("out", [SP, R, T], BF16, kind="ExternalOutput")

    with tile.TileContext(nc) as tc:
        _emit(nc, tc, din, out)
    nc.compile()
    return nc


def _emit(nc, tc, din, out):
    from contextlib import ExitStack

    ctx = ExitStack()
    with ctx:
        # ---------- pools ----------
        const = ctx.enter_context(tc.tile_pool(name="const", bufs=1))
        small = ctx.enter_context(tc.tile_pool(name="small", bufs=1))
        psum = ctx.enter_context(tc.tile_pool(name="psum", bufs=1, space="PSUM"))
        dram = ctx.enter_context(tc.tile_pool(name="dram", bufs=1, space="DRAM"))

        # ---------- pools for the main loop ----------
        CH = 5  # r's per chunk
        CW = CH * T
        # u tiles recycle at ACT pace (pass 3 writes `a` to a separate
        # bf16 tile): the u stream must NOT be gated on the r-loop,
        # which waits for logits, which waits for the wr stream --
        # tying those together starves ACT for the whole wr window.
        upool = ctx.enter_context(tc.tile_pool(name="upool", bufs=3))
        apool = ctx.enter_context(tc.tile_pool(name="apool", bufs=R // CH))
        opool = ctx.enter_context(tc.tile_pool(name="opool", bufs=3))
        qpool = ctx.enter_context(tc.tile_pool(name="qpool", bufs=6))
        spool = ctx.enter_context(tc.tile_pool(name="spool", bufs=4))
        bppool = ctx.enter_context(tc.tile_pool(name="bppool", bufs=5,
                                                space="PSUM"))

        # ---------- load params (HEAD of the sync FIFO: the HWDGE ring
        # drains in emission order, so the GCN/logits critical path must
        # come before the 12.5MB u stream) ----------
        onesb = const.tile([65, 128], BF16, tag="onesb", name="onesb")
        nc.vector.memset(onesb[:], 1.0)

        # pxb first: xw1 (the GCN head) only needs pxb, and every us
        # earlier the GCN starts is a us off the collective-bound tail
        pxb = const.tile([126, 4 * T + 4 * H1], BF16, tag="pxb", name="pxb")
        nc.sync.dma_start(pxb[:], din["pxb"][:])
        pbf = const.tile([125, 8 * T + 4 * R], BF16, tag="pbf", name="pbf")
        nc.sync.dma_start(pbf[:], din["pbf"][:])
        rows = const.tile([1, H1 + H2 + 2 * T], BF16, tag="rows", name="rows")
        nc.sync.dma_start(rows[:], din["rows"][:])
        w2 = const.tile([H1, H2], F32, tag="w2", name="w2")
        nc.sync.dma_start(w2[:], din["w2"][:])
        ident = const.tile([128, 128], F32, tag="ident", name="ident")
        nc.sync.dma_start(ident[:], din["ident"][:])

        adjT = [pbf[:, k * T:(k + 1) * T] for k in range(KT)]
        av = [pbf[:, (KT + k) * T:(KT + k + 1) * T] for k in range(KT)]
        dclT = [pbf[:, 8 * T + k * R:8 * T + (k + 1) * R] for k in range(KT)]
        xT = [pxb[:, k * T:(k + 1) * T] for k in range(KT)]
        w1 = [pxb[:, 4 * T + k * H1:4 * T + (k + 1) * H1] for k in range(KT)]
        b1 = rows[0:1, 0:H1]
        b2 = rows[0:1, H1:H1 + H2]
        grow = rows[0:1, H1 + H2:H1 + H2 + T]
        brow = rows[0:1, H1 + H2 + T:H1 + H2 + 2 * T]

        # first two u chunks ahead of the weight stream so ACT starts
        # at ~13us (ACT consumes u at ~175GB/s = the whole DMA ceiling;
        # it will starve during the wr window regardless, but an early
        # start overlaps the GCN)
        pre_ut = {}
        for r0 in (0, CH):
            ut = upool.tile([SP, CW], F32, tag="u", name="u")
            nc.sync.dma_start(
                ut[:].rearrange("p (c t) -> p c t", c=CH),
                din["u"][:, r0:r0 + CH, :])
            pre_ut[r0] = ut

        # per-core wr shard (2 channels, 1MB) on the gpsimd ring
        wpool = ctx.enter_context(tc.tile_pool(name="wpool", bufs=2))
        wgs = []
        for g in range(2):
            wt = wpool.tile([125, KT * T], BF16, tag="wr_stream",
                            name="wr_stream")
            nc.gpsimd.dma_start(wt[:], din["wr"][g])
            wgs.append(wt)

        # ---------- GCN, transposed formulation ----------
        # bn is folded into the adjacency on the host (adjT ships
        # gamma[t]*adj[t,u] transposed), leaving rank-1 bias terms:
        #   bn(adj@xw+b)^T[c,t] = (xw^T adj1^T)[c,t] + b[c]*gamma[t]
        #                         + beta[t]
        # so each adj product is ONE [H,500] PSUM accumulation of 4
        # K-tiles plus two K=1 bias matmuls, and layer 2 consumes h1T
        # directly as its stationary operand (no transposes, no bn DVE
        # chain).
        def lrelu_from_psum(ps_ap, out_tile, width):
            tmp = small.tile([width, T], F32, tag=f"lr{width}", name=f"lr{width}")
            nc.vector.tensor_scalar_mul(tmp[:], ps_ap, NEG_SLOPE)
            nc.vector.tensor_tensor(out_tile[:], tmp[:], ps_ap,
                                    op=mybir.AluOpType.max)

        xw1 = [small.tile([125, H1], BF16, tag=f"xw1{m}", name=f"xw1{m}") for m in range(KT)]
        for m in range(KT):
            ps = psum.tile([125, H1], F32, tag="ps_small", name="ps_small")
            for k in range(KT):
                nc.tensor.matmul(ps[:], pxb[:, k * T + m * 125:k * T + (m + 1) * 125],
                                 w1[k], start=(k == 0), stop=(k == KT - 1))
            nc.vector.tensor_copy(xw1[m][:], ps[:])

        a1ps = psum.tile([H1, T], F32, tag="ps_small", name="ps_small")
        for k in range(KT):
            nc.tensor.matmul(a1ps[:], xw1[k][:], adjT[k],
                             start=(k == 0), stop=False)
        nc.tensor.matmul(a1ps[:], b1, grow, start=False, stop=False)
        nc.tensor.matmul(a1ps[:], onesb[0:1, :H1], brow, start=False,
                         stop=True)
        h1T = small.tile([H1, T], F32, tag="h1T", name="h1T")
        lrelu_from_psum(a1ps[:], h1T, H1)

        xw2 = [small.tile([125, H2], BF16, tag=f"xw2{m}", name=f"xw2{m}") for m in range(KT)]
        for m in range(KT):
            ps = psum.tile([125, H2], F32, tag="ps_small", name="ps_small")
            nc.tensor.matmul(ps[:], h1T[:, m * 125:(m + 1) * 125], w2[:],
                             start=True, stop=True)
            nc.vector.tensor_copy(xw2[m][:], ps[:])

        a2ps = psum.tile([H2, T], F32, tag="ps_small", name="ps_small")
        for k in range(KT):
            nc.tensor.matmul(a2ps[:], xw2[k][:], adjT[k],
                             start=(k == 0), stop=False)
        nc.tensor.matmul(a2ps[:], b2, grow, start=False, stop=False)
        nc.tensor.matmul(a2ps[:], onesb[0:1, :H2], brow, start=False,
                         stop=True)
        h2T = small.tile([H2, T], F32, tag="h2T", name="h2T")
        lrelu_from_psum(a2ps[:], h2T, H2)

        # h2 back to [t, c] tiles in bf16 for the z matmuls
        h2b = [small.tile([125, H2], BF16, tag=f"h2b{k}", name=f"h2b{k}")
               for k in range(KT)]
        for k in range(KT):
            pt = psum.tile([125, H2], F32, tag="ps_small", name="ps_small")
            nc.tensor.transpose(pt[:], h2T[:, k * 125:(k + 1) * 125],
                                ident[:H2, :H2])
            nc.vector.tensor_copy(h2b[k][:], pt[:])

        # ---------- z partial (2 owned channels) + AllReduce ----------
        zps = psum.tile([1, T], F32, tag="ps_z", name="ps_z")
        first = True
        for c in range(2):
            wt = wgs[c]
            for k in range(KT):
                nc.tensor.matmul(zps[:], h2b[k][:, c:c + 1],
                                 wt[:, k * T:(k + 1) * T],
                                 start=first,
                                 stop=(c == 1 and k == KT - 1))
                first = False
        zpart = small.tile([1, T], F32, tag="zpart", name="zpart")
        nc.vector.tensor_copy(zpart[:], zps[:])
        zin = dram.tile([1, T], F32, name="zin")
        zout = dram.tile([1, T], F32, name="zout")
        # bounce DMAs on the scalar ring: the sync FIFO is full of u
        # chunks and would head-of-line-block these
        nc.scalar.dma_start(zin[:], zpart[:])
        nc.gpsimd.collective_compute(
            "AllReduce", mybir.AluOpType.add,
            replica_groups=[list(range(NCORES))],
            ins=[zin.opt()], outs=[zout.opt()])
        zrow = small.tile([1, T], BF16, tag="zrow", name="zrow")
        nc.gpsimd.dma_start(zrow[:], zout[:])  # SWDGE casts f32->bf16

        # ---------- logits = dcl @ av + z (broadcast over rows) ----------
        lgp = psum.tile([R, T], F32, tag="ps_lg", name="ps_lg")
        for k in range(KT):
            nc.tensor.matmul(lgp[:], dclT[k], av[k],
                             start=(k == 0), stop=False)
        nc.tensor.matmul(lgp[:], onesb[0:1, :R], zrow[:], start=False, stop=True)
        # matmul operands need base partition in {0, 32, 64}; pack the 50
        # L = exp(logits) rows (bf16) into 3 lanes at those partitions,
        # 17 rows each along the free dim. Bounce through DRAM to reshape
        # partitions->free in 3 DMAs.
        LPL = 17  # logits rows per lane
        lgb = small.tile([R, T], BF16, tag="lgb", name="lgb")
        nc.scalar.activation(lgb[:], lgp[:],
                             mybir.ActivationFunctionType.Exp)
        # bounce on the scalar ring: the sync ring is busy streaming u
        # chunks and would head-of-line-block these behind them
        ld = dram.tile([R, T], BF16, name="ldram")
        nc.scalar.dma_start(ld[:], lgb[:])
        fl = small.tile([65, LPL * T], BF16, tag="lgflat", name="lgflat")
        nc.scalar.dma_start(
            fl[0:33:32, :].rearrange("l (j t) -> l j t", j=LPL),
            ld[0:2 * LPL].rearrange("(l j) t -> l j t", l=2))
        nc.scalar.dma_start(fl[64:65, :(R - 2 * LPL) * T],
                            ld[2 * LPL:R].rearrange("(o j) t -> o (j t)", o=1))

        def lg_slice(r):
            lane, j = r // LPL, r % LPL
            return (fl[lane * 32:lane * 32 + 1, j * T:(j + 1) * T],
                    onesb[lane * 32:lane * 32 + 1, :SP])

        # ---------- main sampling loop ----------
        # u is (SP, R, T): each partition (sample) owns a contiguous
        # R*T*4 = 100KB DRAM run. Stream CH r's per chunk so every DMA
        # moves CH*2KB contiguous per partition (large packets), compute
        # a = -1/ln u in 3 chunk-wide in-place ACT passes, then per r:
        # PE-broadcast the L row into PSUM, multiply (+row-sum), then
        # one chunk-wide reciprocal and per-r normalize into bf16.
        for r0 in range(0, R, CH):
            if r0 in pre_ut:
                ut = pre_ut[r0]
            else:
                ut = upool.tile([SP, CW], F32, tag="u", name="u")
                nc.sync.dma_start(
                    ut[:].rearrange("p (c t) -> p c t", c=CH),
                    din["u"][:, r0:r0 + CH, :])
            # a = exp(-ln(-ln u)) = -1/ln(u), three chunk-wide ACT
            # passes (one table set), all independent of the logits.
            # Pass 3 lands in a separate bf16 tile so ut recycles at
            # ACT pace (not r-loop pace).
            nc.scalar.activation(ut[:], ut[:], mybir.ActivationFunctionType.Ln)
            nc.scalar.activation(ut[:], ut[:], mybir.ActivationFunctionType.Ln,
                                 scale=-1.0)
            at = apool.tile([SP, CW], BF16, tag="a", name="a")
            nc.scalar.activation(at[:], ut[:], mybir.ActivationFunctionType.Exp,
                                 scale=-1.0)
            ot = opool.tile([SP, CW], BF16, tag="o", name="o")
            ssc = spool.tile([SP, CH], F32, tag="ss", name="ss")
            rsc = spool.tile([SP, CH], F32, tag="rs", name="rs")
            qts = []
            for g in range(CH):
                seg = slice(g * T, (g + 1) * T)
                # broadcast L row r across partitions via a ones-matmul
                rhs, lhs_ones = lg_slice(r0 + g)
                bt = bppool.tile([SP, 512], F32, tag="bp", name="bp")
                nc.tensor.matmul(bt[:, :T], lhs_ones, rhs,
                                 start=True, stop=True)
                # q = a * L_bcast with fused row-sum
                # (tensor_tensor_reduce fails NEFF-side on this stack;
                # scalar_tensor_tensor with op0=bypass is HW-proven).
                # q goes to a per-r tile, not an ot slice: in-place
                # chains on one chunk tile serialize all 5 r's.
                qt = qpool.tile([SP, T], BF16, tag="q", name="q")
                nc.vector.scalar_tensor_tensor(
                    qt[:], bt[:, :T], 0.0, at[:, seg],
                    op0=mybir.AluOpType.bypass, op1=mybir.AluOpType.mult,
                    accum_out=ssc[:, g:g + 1])
                qts.append(qt)
            nc.vector.reciprocal(rsc[:], ssc[:])
            for g in range(CH):
                seg = slice(g * T, (g + 1) * T)
                # normalize on the Pool engine (idle in the tail) --
                # frees ~10us of DVE on the logits+DVE critical tail
                nc.gpsimd.tensor_scalar_mul(ot[:, seg], qts[g][:],
                                            rsc[:, g:g + 1])
            nc.gpsimd.dma_start(out[:, r0:r0 + CH, :],
                                ot[:].rearrange("p (c t) -> p c t", c=CH))


def _get_nc():
    if "nc" not in _CACHE:
        _CACHE["nc"] = _build()
    return _CACHE["nc"]


def prep_in_maps(inputs):
    import ml_dtypes
    f32 = np.float32
    bf16 = ml_dtypes.bfloat16
    state = np.asarray(inputs["state"], f32)[0]          # (500, 2)
    payoff = np.asarray(inputs["payoff"], f32)           # (500, 500)
    noise = np.asarray(inputs["feat_noise"], f32)[0]     # (500, 2)
    xT = np.concatenate([state, payoff, noise], axis=1).T.copy()  # (504, 500)
    gamma = np.asarray(inputs["bn_gamma"], f32)
    beta = np.asarray(inputs["bn_beta"], f32)
    adjT = (np.asarray(inputs["norm_adj"], f32) * gamma[:, None]).T
    dclT = np.asarray(inputs["def_cur_loc"], f32).T
    wr_full = np.asarray(inputs["actgen_w"], f32).reshape(T, H2, T)
    wr_full = wr_full.transpose(1, 0, 2)                 # (16, 500, 500)
    # per-core 2-channel shards, partition-contiguous:
    # wr_pack[g][c, p, k*T + t] = wr_full[2g + c, k*125 + p, t]
    wr_all = np.ascontiguousarray(
        wr_full.reshape(H2, KT, 125, T).transpose(0, 2, 1, 3)
    ).reshape(H2, 125, KT * T).astype(bf16)
    # mega-packed param planes (see _build)
    adjb = adjT.astype(bf16)    # (500, 500): k-tile rows k*125..
    avb = np.asarray(inputs["actgen_v"], f32).astype(bf16)
    dclb = dclT.astype(bf16)    # (500, 50)
    pbf = np.concatenate(
        [np.concatenate([adjb[k * 125:(k + 1) * 125] for k in range(KT)], axis=1),
         np.concatenate([avb[k * 125:(k + 1) * 125] for k in range(KT)], axis=1),
         np.concatenate([dclb[k * 125:(k + 1) * 125] for k in range(KT)], axis=1)],
        axis=1)                 # (125, 8*500 + 4*50)
    w1f = np.asarray(inputs["gc1_w"], f32)
    pxb = np.concatenate(
        [np.concatenate([xT[k * 126:(k + 1) * 126] for k in range(KT)], axis=1),
         np.concatenate([w1f[k * 126:(k + 1) * 126] for k in range(KT)], axis=1)],
        axis=1).astype(bf16)    # (126, 4*500 + 4*32)
    common = {
        "pbf": np.ascontiguousarray(pbf),
        "pxb": np.ascontiguousarray(pxb),
        "ident": np.eye(128, dtype=f32),
    }
    w2f = np.asarray(inputs["gc2_w"], f32)
    b2f = np.asarray(inputs["gc2_b"], f32).reshape(-1)
    b1f = np.asarray(inputs["gc1_b"], f32).reshape(-1)
    u = np.asarray(inputs["gumbel_u"], f32)              # (1000, 50, 500)
    in_maps = []
    for i in range(NCORES):
        m = dict(common)
        # permute gc2's output channels so this core's z-shard channels
        # (2i, 2i+1) sit at h2[:, 0:2]; channels only feed z, so
        # nothing else changes
        perm = [2 * i, 2 * i + 1] + [c for c in range(H2)
                                     if c not in (2 * i, 2 * i + 1)]
        m["w2"] = np.ascontiguousarray(w2f[:, perm])
        m["rows"] = np.concatenate(
            [b1f, b2f[perm], gamma, beta]).reshape(1, -1).astype(bf16)
        m["wr"] = np.ascontiguousarray(wr_all[2 * i:2 * i + 2])
        m["u"] = np.ascontiguousarray(u[i * SP:(i + 1) * SP])  # (125, 50, 500)
        in_maps.append(m)
    return in_maps


def run(inputs, trace=False):
    nc = _get_nc()
    in_maps = prep_in_maps(inputs)
    res = bass_utils.run_bass_kernel_spmd(
        nc, in_maps, core_ids=list(range(NCORES)), trace=trace)
    full = np.concatenate(
        [np.asarray(res.results[i]["out"]).astype(np.float32)
         for i in range(NCORES)], axis=0)                # (1000, 50, 500)
    return full, res


def kernel(**inputs):
    full, _ = run(inputs)
    return full


# revision 39
# speedup vs baseline: 2.5998x; 2.5998x over previous
"""Trainium2 Bass kernel for nn_Def_A2C_Sample_Generator.

Computation (see reference):
  x = concat(state, payoff, noise)            (500, 504)
  h1 = lrelu(bn(adj @ (x @ w1) + b1))         (500, 32)
  h2 = lrelu(bn(adj @ (h1 @ w2) + b2))        (500, 16)
  xf = h2.reshape(8000)
  logits = xf @ actgen_w + def_cur_loc @ actgen_v          (50, 500)
  out = softmax(logits[None] + gumbel(u), axis=-1)         (1000, 50, 500)

Sharding: data-parallel over the 1000 samples, 125 per core on 8
cores. Each core computes the logits redundantly (small GCN; the 8 MB
bf16 actgen_w is streamed) and softmaxes its own 125 x 50 x 500 gumbel
block.

Softmax is factored so every ACT pass is independent of the logits
(the logits path is the serial prologue; all gumbel work front-runs
it):
  exp(logits + g) with g = -ln(-ln u) equals L * a where
  L = exp(logits) (prologue, 50x500) and a = exp(-ln(-ln u)) = -1/ln u.
Main loop, 5-r chunks in the natural (sample, r, T) layout:
  a       : 3 chunk-wide in-place ACT passes (Ln, Ln(-x), Exp(-x);
            one table set - see the act-table monkeypatch below)
  L bcast : per-r PE ones-matmul, single bf16 plane into f32 PSUM
  q, S    : DVE scalar_tensor_tensor mult with fused row-sum accum
  out     : one DVE reciprocal per chunk + per-r tensor_scalar mult
            into a bf16 chunk tile, one 625KB store per chunk
            (host upcasts to f32; bf16 rounding is ~2e-3 rel, the
            harness gate is 2e-2)

TRN2 PE runs at a ~5x-slow mid p-state unless continuously busy for
3us, and f32 matmuls cost 4 cyc/row on top -- so every prologue
matmul operand that tolerates bf16 (adjT, av, dclT, bias rows, zrow,
xw tiles) is bf16, and the per-r broadcast is one bf16 plane (~0.2%
rounding on L, well inside the error budget).

DMA queues: u loads + params on the sync HWDGE ring (NOT the scalar
ring: HWDGE descriptor generation executes on the issuing engine, and
the scalar ring would burn ~40us of ACT engine time), actgen_w stream
+ output stores on the gpsimd SWDGE path. actgen_w is host-packed
per-partition-contiguous ([16,125,2000]) so each channel load is 125
4KB-run descriptors instead of 500 1KB ones.

Logits rows are packed into 3 lanes at base partitions 0/32/64 (the
only legal matmul operand bases) via a DRAM bounce.
"""
import sys

if "/opt/trn_rl_repo" not in sys.path:
    sys.path.insert(0, "/opt/trn_rl_repo")

import numpy as np

import concourse.bacc as bacc
import concourse.bass as bass
import concourse.mybir as mybir
import concourse.tile as tile
from concourse import bass_utils

# The act-table-load pass resolves Exp -> exp_and_others (id 0) and
# Ln -> natural_log (id 5), thrashing a ~2.7us table swap at every
# Ln<->Exp transition in the main loop. natural_log_exp_and_others
# (id 6) holds BOTH. Hide exp/ln from the other sets in the map the
# chooser reads (ids keep indexing the real act_info.json, so the
# loaded tables are unchanged) so every Exp and Ln lands on set 6 and
# one load suffices.
_orig_get_act_tables = bacc.get_activation_tables


def _patched_get_act_tables(arch):
    tabs = dict(_orig_get_act_tables(arch))
    both = {mybir.ActivationFunctionType.Exp, mybir.ActivationFunctionType.Ln}
    for name, fns in tabs.items():
        if name != "natural_log_exp_and_others" and (both & fns):
            tabs[name] = fns - both
    return tabs


bacc.get_activation_tables = _patched_get_act_tables

F32 = mybir.dt.float32
BF16 = mybir.dt.bfloat16
NCORES = 8
T = 500
R = 50
NS = 1000
SP = NS // NCORES  # 125 samples per core
H1, H2 = 32, 16
FIN = 504  # 2 + 500 + 2 input features
KT = 4  # K/M tiling of the 500 dim into 4x125
NEG_SLOPE = 0.2

_CACHE = {}


def _build():
    nc = bacc.Bacc("TRN2", target_bir_lowering=False, debug=False,
                   enable_asserts=False, num_devices=NCORES)

    # ---- I/O ----
    din = {}
    # mega-packed params: per-DMA fixed costs on the HWDGE ring are
    # ~1-2us each, so ~25 small tile loads serialize into ~45us. Two
    # packed planes load in ~8us instead.
    # pbf[p, :] = adjT k-tiles (4x500) | av k-tiles (4x500) | dclT (4x50)
    din["pbf"] = nc.dram_tensor("pbf", [125, 8 * T + 4 * R], BF16,
                                kind="ExternalInput")
    # pxb[p, :] = xT k-tiles (4x500) | w1 k-tiles (4x32), bf16
    din["pxb"] = nc.dram_tensor("pxb", [126, 4 * T + 4 * H1], BF16,
                                kind="ExternalInput")
    # rows[0, :] = b1 (32) | b2 (16) | grow (500) | brow (500)
    din["rows"] = nc.dram_tensor("rows", [1, H1 + H2 + 2 * T], BF16,
                                 kind="ExternalInput")
    din["w2"] = nc.dram_tensor("w2", [H1, H2], F32, kind="ExternalInput")
    # per-core actgen_w shard: 2 of 16 channels (the kernel is HBM-byte
    # bound at ~133GB/s/core, so the 8MB replicated stream IS the
    # bottleneck; each core computes a partial z from its 2 channels
    # and a 2KB ncfw AllReduce -- whose ~tens-of-us latency is fully
    # hidden, DVE only needs logits at ~110us -- completes it).
    # Host-side, gc2's output channels are permuted per core so the
    # owned channels sit at h2[:, 0:2]; channels only feed z.
    din["wr"] = nc.dram_tensor("wr", [2, 125, KT * T], BF16,
                               kind="ExternalInput")
    din["ident"] = nc.dram_tensor("ident", [128, 128], F32, kind="ExternalInput")
    din["u"] = nc.dram_tensor("u", [SP, R, T], F32, kind="ExternalInput")
    out = nc.dram_tensor("out", [SP, R, T], BF16, kind="ExternalOutput")

    with tile.TileContext(nc) as tc:
        _emit(nc, tc, din, out)
    nc.compile()
    return nc


def _emit(nc, tc, din, out):
    from contextlib import ExitStack

    ctx = ExitStack()
    with ctx:
        # ---------- pools ----------
        const = ctx.enter_context(tc.tile_pool(name="const", bufs=1))
        small = ctx.enter_context(tc.tile_pool(name="small", bufs=1))
        psum = ctx.enter_context(tc.tile_pool(name="psum", bufs=1, space="PSUM"))
        dram = ctx.enter_context(tc.tile_pool(name="dram", bufs=1, space="DRAM"))

        # ---------- pools for the main loop ----------
        CH = 5  # r's per chunk
        CW = CH * T
        # u tiles recycle at ACT pace (pass 3 writes `a` to a separate
        # bf16 tile): the u stream must NOT be gated on the r-loop,
        # which waits for logits, which waits for the wr stream --
        # tying those together starves ACT for the whole wr window.
        upool = ctx.enter_context(tc.tile_pool(name="upool", bufs=3))
        apool = ctx.enter_context(tc.tile_pool(name="apool", bufs=R // CH))
        opool = ctx.enter_context(tc.tile_pool(name="opool", bufs=3))
        bcpool = ctx.enter_context(tc.tile_pool(name="bcpool", bufs=6))
        qpool = ctx.enter_context(tc.tile_pool(name="qpool", bufs=6))
        spool = ctx.enter_context(tc.tile_pool(name="spool", bufs=4))
        bppool = ctx.enter_context(tc.tile_pool(name="bppool", bufs=5,
                                                space="PSUM"))

        # ---------- load params (HEAD of the sync FIFO: the HWDGE ring
        # drains in emission order, so the GCN/logits critical path must
        # come before the 12.5MB u stream) ----------
        onesb = const.tile([65, 128], BF16, tag="onesb", name="onesb")
        nc.vector.memset(onesb[:], 1.0)

        # pxb first: xw1 (the GCN head) only needs pxb, and every us
        # earlier the GCN starts is a us off the collective-bound tail
        pxb = const.tile([126, 4 * T + 4 * H1], BF16, tag="pxb", name="pxb")
        nc.sync.dma_start(pxb[:], din["pxb"][:])
        pbf = const.tile([125, 8 * T + 4 * R], BF16, tag="pbf", name="pbf")
        nc.sync.dma_start(pbf[:], din["pbf"][:])
        rows = const.tile([1, H1 + H2 + 2 * T], BF16, tag="rows", name="rows")
        nc.sync.dma_start(rows[:], din["rows"][:])
        w2 = const.tile([H1, H2], F32, tag="w2", name="w2")
        nc.sync.dma_start(w2[:], din["w2"][:])
        ident = const.tile([128, 128], F32, tag="ident", name="ident")
        nc.sync.dma_start(ident[:], din["ident"][:])

        adjT = [pbf[:, k * T:(k + 1) * T] for k in range(KT)]
        av = [pbf[:, (KT + k) * T:(KT + k + 1) * T] for k in range(KT)]
        dclT = [pbf[:, 8 * T + k * R:8 * T + (k + 1) * R] for k in range(KT)]
        xT = [pxb[:, k * T:(k + 1) * T] for k in range(KT)]
        w1 = [pxb[:, 4 * T + k * H1:4 * T + (k + 1) * H1] for k in range(KT)]
        b1 = rows[0:1, 0:H1]
        b2 = rows[0:1, H1:H1 + H2]
        grow = rows[0:1, H1 + H2:H1 + H2 + T]
        brow = rows[0:1, H1 + H2 + T:H1 + H2 + 2 * T]

        # first two u chunks ahead of the weight stream so ACT starts
        # at ~13us (ACT consumes u at ~175GB/s = the whole DMA ceiling;
        # it will starve during the wr window regardless, but an early
        # start overlaps the GCN)
        pre_ut = {}
        for r0 in (0, CH):
            ut = upool.tile([SP, CW], F32, tag="u", name="u")
            nc.sync.dma_start(
                ut[:].rearrange("p (c t) -> p c t", c=CH),
                din["u"][:, r0:r0 + CH, :])
            pre_ut[r0] = ut

        # per-core wr shard (2 channels, 1MB) on the gpsimd ring
        wpool = ctx.enter_context(tc.tile_pool(name="wpool", bufs=2))
        wgs = []
        for g in range(2):
            wt = wpool.tile([125, KT * T], BF16, tag="wr_stream",
                            name="wr_stream")
            nc.gpsimd.dma_start(wt[:], din["wr"][g])
            wgs.append(wt)

        # ---------- GCN, transposed formulation ----------
        # bn is folded into the adjacency on the host (adjT ships
        # gamma[t]*adj[t,u] transposed), leaving rank-1 bias terms:
        #   bn(adj@xw+b)^T[c,t] = (xw^T adj1^T)[c,t] + b[c]*gamma[t]
        #                         + beta[t]
        # so each adj product is ONE [H,500] PSUM accumulation of 4
        # K-tiles plus two K=1 bias matmuls, and layer 2 consumes h1T
        # directly as its stationary operand (no transposes, no bn DVE
        # chain).
        def lrelu_from_psum(ps_ap, out_tile, width):
            tmp = small.tile([width, T], F32, tag=f"lr{width}", name=f"lr{width}")
            nc.vector.tensor_scalar_mul(tmp[:], ps_ap, NEG_SLOPE)
            nc.vector.tensor_tensor(out_tile[:], tmp[:], ps_ap,
                                    op=mybir.AluOpType.max)

        xw1 = [small.tile([125, H1], BF16, tag=f"xw1{m}", name=f"xw1{m}") for m in range(KT)]
        for m in range(KT):
            ps = psum.tile([125, H1], F32, tag="ps_small", name="ps_small")
            for k in range(KT):
                nc.tensor.matmul(ps[:], pxb[:, k * T + m * 125:k * T + (m + 1) * 125],
                                 w1[k], start=(k == 0), stop=(k == KT - 1))
            nc.vector.tensor_copy(xw1[m][:], ps[:])

        a1ps = psum.tile([H1, T], F32, tag="ps_small", name="ps_small")
        for k in range(KT):
            nc.tensor.matmul(a1ps[:], xw1[k][:], adjT[k],
                             start=(k == 0), stop=False)
        nc.tensor.matmul(a1ps[:], b1, grow, start=False, stop=False)
        nc.tensor.matmul(a1ps[:], onesb[0:1, :H1], brow, start=False,
                         stop=True)
        h1T = small.tile([H1, T], F32, tag="h1T", name="h1T")
        lrelu_from_psum(a1ps[:], h1T, H1)

        xw2 = [small.tile([125, H2], BF16, tag=f"xw2{m}", name=f"xw2{m}") for m in range(KT)]
        for m in range(KT):
            ps = psum.tile([125, H2], F32, tag="ps_small", name="ps_small")
            nc.tensor.matmul(ps[:], h1T[:, m * 125:(m + 1) * 125], w2[:],
                             start=True, stop=True)
            nc.vector.tensor_copy(xw2[m][:], ps[:])

        a2ps = psum.tile([H2, T], F32, tag="ps_small", name="ps_small")
        for k in range(KT):
            nc.tensor.matmul(a2ps[:], xw2[k][:], adjT[k],
                             start=(k == 0), stop=False)
        nc.tensor.matmul(a2ps[:], b2, grow, start=False, stop=False)
        nc.tensor.matmul(a2ps[:], onesb[0:1, :H2], brow, start=False,
                         stop=True)
        h2T = small.tile([H2, T], F32, tag="h2T", name="h2T")
        lrelu_from_psum(a2ps[:], h2T, H2)

        # h2 back to [t, c] tiles in bf16 for the z matmuls
        h2b = [small.tile([125, H2], BF16, tag=f"h2b{k}", name=f"h2b{k}")
               for k in range(KT)]
        for k in range(KT):
            pt = psum.tile([125, H2], F32, tag="ps_small", name="ps_small")
            nc.tensor.transpose(pt[:], h2T[:, k * 125:(k + 1) * 125],
                                ident[:H2, :H2])
            nc.vector.tensor_copy(h2b[k][:], pt[:])

        # ---------- z partial (2 owned channels) + AllReduce ----------
        zps = psum.tile([1, T], F32, tag="ps_z", name="ps_z")
        first = True
        for c in range(2):
            wt = wgs[c]
            for k in range(KT):
                nc.tensor.matmul(zps[:], h2b[k][:, c:c + 1],
                                 wt[:, k * T:(k + 1) * T],
                                 start=first,
                                 stop=(c == 1 and k == KT - 1))
                first = False
        zpart = small.tile([1, T], F32, tag="zpart", name="zpart")
        nc.vector.tensor_copy(zpart[:], zps[:])
        zin = dram.tile([1, T], F32, name="zin")
        zout = dram.tile([1, T], F32, name="zout")
        # bounce DMAs on the scalar ring: the sync FIFO is full of u
        # chunks and would head-of-line-block these
        nc.scalar.dma_start(zin[:], zpart[:])
        nc.gpsimd.collective_compute(
            "AllReduce", mybir.AluOpType.add,
            replica_groups=[list(range(NCORES))],
            ins=[zin.opt()], outs=[zout.opt()])
        zrow = small.tile([1, T], BF16, tag="zrow", name="zrow")
        nc.gpsimd.dma_start(zrow[:], zout[:])  # SWDGE casts f32->bf16

        # ---------- logits = dcl @ av + z (broadcast over rows) ----------
        lgp = psum.tile([R, T], F32, tag="ps_lg", name="ps_lg")
        for k in range(KT):
            nc.tensor.matmul(lgp[:], dclT[k], av[k],
                             start=(k == 0), stop=False)
        nc.tensor.matmul(lgp[:], onesb[0:1, :R], zrow[:], start=False, stop=True)
        # matmul operands need base partition in {0, 32, 64}; pack the 50
        # L = exp(logits) rows (bf16) into 3 lanes at those partitions,
        # 17 rows each along the free dim. Bounce through DRAM to reshape
        # partitions->free in 3 DMAs.
        LPL = 17  # logits rows per lane
        lgb = small.tile([R, T], BF16, tag="lgb", name="lgb")
        nc.scalar.activation(lgb[:], lgp[:],
                             mybir.ActivationFunctionType.Exp)
        # bounce on the scalar ring: the sync ring is busy streaming u
        # chunks and would head-of-line-block these behind them
        ld = dram.tile([R, T], BF16, name="ldram")
        nc.scalar.dma_start(ld[:], lgb[:])
        fl = small.tile([65, LPL * T], BF16, tag="lgflat", name="lgflat")
        nc.scalar.dma_start(
            fl[0:33:32, :].rearrange("l (j t) -> l j t", j=LPL),
            ld[0:2 * LPL].rearrange("(l j) t -> l j t", l=2))
        nc.scalar.dma_start(fl[64:65, :(R - 2 * LPL) * T],
                            ld[2 * LPL:R].rearrange("(o j) t -> o (j t)", o=1))

        def lg_slice(r):
            lane, j = r // LPL, r % LPL
            return (fl[lane * 32:lane * 32 + 1, j * T:(j + 1) * T],
                    onesb[lane * 32:lane * 32 + 1, :SP])

        # ---------- main sampling loop ----------
        # u is (SP, R, T): each partition (sample) owns a contiguous
        # R*T*4 = 100KB DRAM run. Stream CH r's per chunk so every DMA
        # moves CH*2KB contiguous per partition (large packets), compute
        # a = -1/ln u in 3 chunk-wide in-place ACT passes, then per r:
        # PE-broadcast the L row into PSUM, multiply (+row-sum), then
        # one chunk-wide reciprocal and per-r normalize into bf16.
        for r0 in range(0, R, CH):
            if r0 in pre_ut:
                ut = pre_ut[r0]
            else:
                ut = upool.tile([SP, CW], F32, tag="u", name="u")
                nc.sync.dma_start(
                    ut[:].rearrange("p (c t) -> p c t", c=CH),
                    din["u"][:, r0:r0 + CH, :])
            # a = exp(-ln(-ln u)) = -1/ln(u), three chunk-wide ACT
            # passes (one table set), all independent of the logits.
            # Pass 3 lands in a separate bf16 tile so ut recycles at
            # ACT pace (not r-loop pace).
            nc.scalar.activation(ut[:], ut[:], mybir.ActivationFunctionType.Ln)
            nc.scalar.activation(ut[:], ut[:], mybir.ActivationFunctionType.Ln,
                                 scale=-1.0)
            at = apool.tile([SP, CW], BF16, tag="a", name="a")
            nc.scalar.activation(at[:], ut[:], mybir.ActivationFunctionType.Exp,
                                 scale=-1.0)
            ot = opool.tile([SP, CW], BF16, tag="o", name="o")
            ssc = spool.tile([SP, CH], F32, tag="ss", name="ss")
            rsc = spool.tile([SP, CH], F32, tag="rs", name="rs")
            qts = []
            for g in range(CH):
                seg = slice(g * T, (g + 1) * T)
                # broadcast L row r across partitions via a ones-matmul
                rhs, lhs_ones = lg_slice(r0 + g)
                bt = bppool.tile([SP, 512], F32, tag="bp", name="bp")
                nc.tensor.matmul(bt[:, :T], lhs_ones, rhs,
                                 start=True, stop=True)
                # q = a * L_bcast with fused row-sum
                # (tensor_tensor_reduce fails NEFF-side on this stack;
                # scalar_tensor_tensor with op0=bypass is HW-proven).
                # q goes to a per-r tile, not an ot slice: in-place
                # chains on one chunk tile serialize all 5 r's.
                qt = qpool.tile([SP, T], BF16, tag="q", name="q")
                nc.vector.scalar_tensor_tensor(
                    qt[:], bt[:, :T], 0.0, at[:, seg],
                    op0=mybir.AluOpType.bypass, op1=mybir.AluOpType.mult,
                    accum_out=ssc[:, g:g + 1])
                qts.append(qt)
            nc.vector.reciprocal(rsc[:], ssc[:])
            for g in range(CH):
                seg = slice(g * T, (g + 1) * T)
                # (keep on DVE: a Pool-engine tensor_scalar_mul here
                # measured 3x WORSE end-to-end)
                nc.vector.tensor_scalar_mul(ot[:, seg], qts[g][:],
                                            rsc[:, g:g + 1])
            nc.gpsimd.dma_start(out[:, r0:r0 + CH, :],
                                ot[:].rearrange("p (c t) -> p c t", c=CH))


def _get_nc():
    if "nc" not in _CACHE:
        _CACHE["nc"] = _build()
    return _CACHE["nc"]


def prep_in_maps(inputs):
    import ml_dtypes
    f32 = np.float32
    bf16 = ml_dtypes.bfloat16
    state = np.asarray(inputs["state"], f32)[0]          # (500, 2)
    payoff = np.asarray(inputs["payoff"], f32)           # (500, 500)
    noise = np.asarray(inputs["feat_noise"], f32)[0]     # (500, 2)
    xT = np.concatenate([state, payoff, noise], axis=1).T.copy()  # (504, 500)
    gamma = np.asarray(inputs["bn_gamma"], f32)
    beta = np.asarray(inputs["bn_beta"], f32)
    adjT = (np.asarray(inputs["norm_adj"], f32) * gamma[:, None]).T
    dclT = np.asarray(inputs["def_cur_loc"], f32).T
    wr_full = np.asarray(inputs["actgen_w"], f32).reshape(T, H2, T)
    wr_full = wr_full.transpose(1, 0, 2)                 # (16, 500, 500)
    # per-core 2-channel shards, partition-contiguous:
    # wr_pack[g][c, p, k*T + t] = wr_full[2g + c, k*125 + p, t]
    wr_all = np.ascontiguousarray(
        wr_full.reshape(H2, KT, 125, T).transpose(0, 2, 1, 3)
    ).reshape(H2, 125, KT * T).astype(bf16)
    # mega-packed param planes (see _build)
    adjb = adjT.astype(bf16)    # (500, 500): k-tile rows k*125..
    avb = np.asarray(inputs["actgen_v"], f32).astype(bf16)
    dclb = dclT.astype(bf16)    # (500, 50)
    pbf = np.concatenate(
        [np.concatenate([adjb[k * 125:(k + 1) * 125] for k in range(KT)], axis=1),
         np.concatenate([avb[k * 125:(k + 1) * 125] for k in range(KT)], axis=1),
         np.concatenate([dclb[k * 125:(k + 1) * 125] for k in range(KT)], axis=1)],
        axis=1)                 # (125, 8*500 + 4*50)
    w1f = np.asarray(inputs["gc1_w"], f32)
    pxb = np.concatenate(
        [np.concatenate([xT[k * 126:(k + 1) * 126] for k in range(KT)], axis=1),
         np.concatenate([w1f[k * 126:(k + 1) * 126] for k in range(KT)], axis=1)],
        axis=1).astype(bf16)    # (126, 4*500 + 4*32)
    common = {
        "pbf": np.ascontiguousarray(pbf),
        "pxb": np.ascontiguousarray(pxb),
        "ident": np.eye(128, dtype=f32),
    }
    w2f = np.asarray(inputs["gc2_w"], f32)
    b2f = np.asarray(inputs["gc2_b"], f32).reshape(-1)
    b1f = np.asarray(inputs["gc1_b"], f32).reshape(-1)
    u = np.asarray(inputs["gumbel_u"], f32)              # (1000, 50, 500)
    in_maps = []
    for i in range(NCORES):
        m = dict(common)
        # permute gc2's output channels so this core's z-shard channels
        # (2i, 2i+1) sit at h2[:, 0:2]; channels only feed z, so
        # nothing else changes
        perm = [2 * i, 2 * i + 1] + [c for c in range(H2)
                                     if c not in (2 * i, 2 * i + 1)]
        m["w2"] = np.ascontiguousarray(w2f[:, perm])
        m["rows"] = np.concatenate(
            [b1f, b2f[perm], gamma, beta]).reshape(1, -1).astype(bf16)
        m["wr"] = np.ascontiguousarray(wr_all[2 * i:2 * i + 2])
        m["u"] = np.ascontiguousarray(u[i * SP:(i + 1) * SP])  # (125, 50, 500)
        in_maps.append(m)
    return in_maps


def run(inputs, trace=False):
    nc = _get_nc()
    in_maps = prep_in_maps(inputs)
    res = bass_utils.run_bass_kernel_spmd(
        nc, in_maps, core_ids=list(range(NCORES)), trace=trace)
    full = np.concatenate(
        [np.asarray(res.results[i]["out"]).astype(np.float32)
         for i in range(NCORES)], axis=0)                # (1000, 50, 500)
    return full, res


def kernel(**inputs):
    full, _ = run(inputs)
    return full


# revision 43
# speedup vs baseline: 2.6391x; 1.0151x over previous
"""Trainium2 Bass kernel for nn_Def_A2C_Sample_Generator.

Computation (see reference):
  x = concat(state, payoff, noise)            (500, 504)
  h1 = lrelu(bn(adj @ (x @ w1) + b1))         (500, 32)
  h2 = lrelu(bn(adj @ (h1 @ w2) + b2))        (500, 16)
  xf = h2.reshape(8000)
  logits = xf @ actgen_w + def_cur_loc @ actgen_v          (50, 500)
  out = softmax(logits[None] + gumbel(u), axis=-1)         (1000, 50, 500)

Sharding: data-parallel over the 1000 samples, 125 per core on 8
cores. Each core computes the logits redundantly (small GCN; the 8 MB
bf16 actgen_w is streamed) and softmaxes its own 125 x 50 x 500 gumbel
block.

Softmax is factored so every ACT pass is independent of the logits
(the logits path is the serial prologue; all gumbel work front-runs
it):
  exp(logits + g) with g = -ln(-ln u) equals L * a where
  L = exp(logits) (prologue, 50x500) and a = exp(-ln(-ln u)) = -1/ln u.
Main loop, 5-r chunks in the natural (sample, r, T) layout:
  a       : 3 chunk-wide in-place ACT passes (Ln, Ln(-x), Exp(-x);
            one table set - see the act-table monkeypatch below)
  L bcast : per-r PE ones-matmul, single bf16 plane into f32 PSUM
  q, S    : DVE scalar_tensor_tensor mult with fused row-sum accum
  out     : one DVE reciprocal per chunk + per-r tensor_scalar mult
            into a bf16 chunk tile, one 625KB store per chunk
            (host upcasts to f32; bf16 rounding is ~2e-3 rel, the
            harness gate is 2e-2)

TRN2 PE runs at a ~5x-slow mid p-state unless continuously busy for
3us, and f32 matmuls cost 4 cyc/row on top -- so every prologue
matmul operand that tolerates bf16 (adjT, av, dclT, bias rows, zrow,
xw tiles) is bf16, and the per-r broadcast is one bf16 plane (~0.2%
rounding on L, well inside the error budget).

DMA queues: u loads + params on the sync HWDGE ring (NOT the scalar
ring: HWDGE descriptor generation executes on the issuing engine, and
the scalar ring would burn ~40us of ACT engine time), actgen_w stream
+ output stores on the gpsimd SWDGE path. actgen_w is host-packed
per-partition-contiguous ([16,125,2000]) so each channel load is 125
4KB-run descriptors instead of 500 1KB ones.

Logits rows are packed into 3 lanes at base partitions 0/32/64 (the
only legal matmul operand bases) via a DRAM bounce.
"""
import sys

if "/opt/trn_rl_repo" not in sys.path:
    sys.path.insert(0, "/opt/trn_rl_repo")

import numpy as np

import concourse.bacc as bacc
import concourse.bass as bass
import concourse.mybir as mybir
import concourse.tile as tile
from concourse import bass_utils

# The act-table-load pass resolves Exp -> exp_and_others (id 0) and
# Ln -> natural_log (id 5), thrashing a ~2.7us table swap at every
# Ln<->Exp transition in the main loop. natural_log_exp_and_others
# (id 6) holds BOTH. Hide exp/ln from the other sets in the map the
# chooser reads (ids keep indexing the real act_info.json, so the
# loaded tables are unchanged) so every Exp and Ln lands on set 6 and
# one load suffices.
_orig_get_act_tables = bacc.get_activation_tables


def _patched_get_act_tables(arch):
    tabs = dict(_orig_get_act_tables(arch))
    both = {mybir.ActivationFunctionType.Exp, mybir.ActivationFunctionType.Ln}
    for name, fns in tabs.items():
        if name != "natural_log_exp_and_others" and (both & fns):
            tabs[name] = fns - both
    return tabs


bacc.get_activation_tables = _patched_get_act_tables

F32 = mybir.dt.float32
BF16 = mybir.dt.bfloat16
NCORES = 8
T = 500
R = 50
NS = 1000
SP = NS // NCORES  # 125 samples per core
H1, H2 = 32, 16
FIN = 504  # 2 + 500 + 2 input features
KT = 4  # K/M tiling of the 500 dim into 4x125
NEG_SLOPE = 0.2

_CACHE = {}


def _build():
    nc = bacc.Bacc("TRN2", target_bir_lowering=False, debug=False,
                   enable_asserts=False, num_devices=NCORES)

    # ---- I/O ----
    din = {}
    # mega-packed params: per-DMA fixed costs on the HWDGE ring are
    # ~1-2us each, so ~25 small tile loads serialize into ~45us. Two
    # packed planes load in ~8us instead.
    # pbf[p, :] = adjT k-tiles (4x500) | av k-tiles (4x500) | dclT (4x50)
    din["pbf"] = nc.dram_tensor("pbf", [125, 8 * T + 4 * R], BF16,
                                kind="ExternalInput")
    # pxb[p, :] = xT k-tiles (4x500) | w1 k-tiles (4x32), bf16
    din["pxb"] = nc.dram_tensor("pxb", [126, 4 * T + 4 * H1], BF16,
                                kind="ExternalInput")
    # rows[0, :] = b1 (32) | b2 (16) | grow (500) | brow (500)
    din["rows"] = nc.dram_tensor("rows", [1, H1 + H2 + 2 * T], BF16,
                                 kind="ExternalInput")
    din["w2"] = nc.dram_tensor("w2", [H1, H2], F32, kind="ExternalInput")
    # per-core actgen_w shard: 2 of 16 channels (the kernel is HBM-byte
    # bound at ~133GB/s/core, so the 8MB replicated stream IS the
    # bottleneck; each core computes a partial z from its 2 channels
    # and a 2KB ncfw AllReduce -- whose ~tens-of-us latency is fully
    # hidden, DVE only needs logits at ~110us -- completes it).
    # Host-side, gc2's output channels are permuted per core so the
    # owned channels sit at h2[:, 0:2]; channels only feed z.
    din["wr"] = nc.dram_tensor("wr", [2, 125, KT * T], BF16,
                               kind="ExternalInput")
    din["ident"] = nc.dram_tensor("ident", [128, 128], F32, kind="ExternalInput")
    din["u"] = nc.dram_tensor("u", [SP, R, T], F32, kind="ExternalInput")
    out = nc.dram_tensor("out", [SP, R, T], BF16, kind="ExternalOutput")

    with tile.TileContext(nc) as tc:
        _emit(nc, tc, din, out)
    nc.compile()
    return nc


def _emit(nc, tc, din, out):
    from contextlib import ExitStack

    ctx = ExitStack()
    with ctx:
        # ---------- pools ----------
        const = ctx.enter_context(tc.tile_pool(name="const", bufs=1))
        small = ctx.enter_context(tc.tile_pool(name="small", bufs=1))
        psum = ctx.enter_context(tc.tile_pool(name="psum", bufs=1, space="PSUM"))
        dram = ctx.enter_context(tc.tile_pool(name="dram", bufs=1, space="DRAM"))

        # ---------- pools for the main loop ----------
        CH = 5  # r's per chunk
        CW = CH * T
        # u tiles recycle at ACT pace (pass 3 writes `a` to a separate
        # bf16 tile): the u stream must NOT be gated on the r-loop,
        # which waits for logits, which waits for the wr stream --
        # tying those together starves ACT for the whole wr window.
        upool = ctx.enter_context(tc.tile_pool(name="upool", bufs=3))
        apool = ctx.enter_context(tc.tile_pool(name="apool", bufs=R // CH))
        opool = ctx.enter_context(tc.tile_pool(name="opool", bufs=3))
        qpool = ctx.enter_context(tc.tile_pool(name="qpool", bufs=6))
        spool = ctx.enter_context(tc.tile_pool(name="spool", bufs=4))
        bppool = ctx.enter_context(tc.tile_pool(name="bppool", bufs=5,
                                                space="PSUM"))

        # ---------- load params (HEAD of the sync FIFO: the HWDGE ring
        # drains in emission order, so the GCN/logits critical path must
        # come before the 12.5MB u stream) ----------
        onesb = const.tile([65, 128], BF16, tag="onesb", name="onesb")
        nc.vector.memset(onesb[:], 1.0)

        pbf = const.tile([125, 8 * T + 4 * R], BF16, tag="pbf", name="pbf")
        nc.sync.dma_start(pbf[:], din["pbf"][:])
        pxb = const.tile([126, 4 * T + 4 * H1], BF16, tag="pxb", name="pxb")
        nc.sync.dma_start(pxb[:], din["pxb"][:])
        rows = const.tile([1, H1 + H2 + 2 * T], BF16, tag="rows", name="rows")
        nc.sync.dma_start(rows[:], din["rows"][:])
        w2 = const.tile([H1, H2], F32, tag="w2", name="w2")
        nc.sync.dma_start(w2[:], din["w2"][:])
        ident = const.tile([128, 128], F32, tag="ident", name="ident")
        nc.sync.dma_start(ident[:], din["ident"][:])

        adjT = [pbf[:, k * T:(k + 1) * T] for k in range(KT)]
        av = [pbf[:, (KT + k) * T:(KT + k + 1) * T] for k in range(KT)]
        dclT = [pbf[:, 8 * T + k * R:8 * T + (k + 1) * R] for k in range(KT)]
        xT = [pxb[:, k * T:(k + 1) * T] for k in range(KT)]
        w1 = [pxb[:, 4 * T + k * H1:4 * T + (k + 1) * H1] for k in range(KT)]
        b1 = rows[0:1, 0:H1]
        b2 = rows[0:1, H1:H1 + H2]
        grow = rows[0:1, H1 + H2:H1 + H2 + T]
        brow = rows[0:1, H1 + H2 + T:H1 + H2 + 2 * T]

        # first two u chunks ahead of the weight stream so ACT starts
        # at ~13us (ACT consumes u at ~175GB/s = the whole DMA ceiling;
        # it will starve during the wr window regardless, but an early
        # start overlaps the GCN)
        pre_ut = {}
        for r0 in (0, CH):
            ut = upool.tile([SP, CW], F32, tag="u", name="u")
            nc.sync.dma_start(
                ut[:].rearrange("p (c t) -> p c t", c=CH),
                din["u"][:, r0:r0 + CH, :])
            pre_ut[r0] = ut

        # per-core wr shard (2 channels, 1MB) on the gpsimd ring
        wpool = ctx.enter_context(tc.tile_pool(name="wpool", bufs=2))
        wgs = []
        for g in range(2):
            wt = wpool.tile([125, KT * T], BF16, tag="wr_stream",
                            name="wr_stream")
            nc.gpsimd.dma_start(wt[:], din["wr"][g])
            wgs.append(wt)

        # ---------- GCN, transposed formulation ----------
        # bn is folded into the adjacency on the host (adjT ships
        # gamma[t]*adj[t,u] transposed), leaving rank-1 bias terms:
        #   bn(adj@xw+b)^T[c,t] = (xw^T adj1^T)[c,t] + b[c]*gamma[t]
        #                         + beta[t]
        # so each adj product is ONE [H,500] PSUM accumulation of 4
        # K-tiles plus two K=1 bias matmuls, and layer 2 consumes h1T
        # directly as its stationary operand (no transposes, no bn DVE
        # chain).
        def lrelu_from_psum(ps_ap, out_tile, width):
            tmp = small.tile([width, T], F32, tag=f"lr{width}", name=f"lr{width}")
            nc.vector.tensor_scalar_mul(tmp[:], ps_ap, NEG_SLOPE)
            nc.vector.tensor_tensor(out_tile[:], tmp[:], ps_ap,
                                    op=mybir.AluOpType.max)

        xw1 = [small.tile([125, H1], BF16, tag=f"xw1{m}", name=f"xw1{m}") for m in range(KT)]
        for m in range(KT):
            ps = psum.tile([125, H1], F32, tag="ps_small", name="ps_small")
            for k in range(KT):
                nc.tensor.matmul(ps[:], pxb[:, k * T + m * 125:k * T + (m + 1) * 125],
                                 w1[k], start=(k == 0), stop=(k == KT - 1))
            nc.vector.tensor_copy(xw1[m][:], ps[:])

        a1ps = psum.tile([H1, T], F32, tag="ps_small", name="ps_small")
        for k in range(KT):
            nc.tensor.matmul(a1ps[:], xw1[k][:], adjT[k],
                             start=(k == 0), stop=False)
        nc.tensor.matmul(a1ps[:], b1, grow, start=False, stop=False)
        nc.tensor.matmul(a1ps[:], onesb[0:1, :H1], brow, start=False,
                         stop=True)
        h1T = small.tile([H1, T], F32, tag="h1T", name="h1T")
        lrelu_from_psum(a1ps[:], h1T, H1)

        xw2 = [small.tile([125, H2], BF16, tag=f"xw2{m}", name=f"xw2{m}") for m in range(KT)]
        for m in range(KT):
            ps = psum.tile([125, H2], F32, tag="ps_small", name="ps_small")
            nc.tensor.matmul(ps[:], h1T[:, m * 125:(m + 1) * 125], w2[:],
                             start=True, stop=True)
            nc.vector.tensor_copy(xw2[m][:], ps[:])

        a2ps = psum.tile([H2, T], F32, tag="ps_small", name="ps_small")
        for k in range(KT):
            nc.tensor.matmul(a2ps[:], xw2[k][:], adjT[k],
                             start=(k == 0), stop=False)
        nc.tensor.matmul(a2ps[:], b2, grow, start=False, stop=False)
        nc.tensor.matmul(a2ps[:], onesb[0:1, :H2], brow, start=False,
                         stop=True)
        h2T = small.tile([H2, T], F32, tag="h2T", name="h2T")
        lrelu_from_psum(a2ps[:], h2T, H2)

        # h2 back to [t, c] tiles in bf16 for the z matmuls
        h2b = [small.tile([125, H2], BF16, tag=f"h2b{k}", name=f"h2b{k}")
               for k in range(KT)]
        for k in range(KT):
            pt = psum.tile([125, H2], F32, tag="ps_small", name="ps_small")
            nc.tensor.transpose(pt[:], h2T[:, k * 125:(k + 1) * 125],
                                ident[:H2, :H2])
            nc.vector.tensor_copy(h2b[k][:], pt[:])

        # ---------- z partial (2 owned channels) + AllReduce ----------
        zps = psum.tile([1, T], F32, tag="ps_z", name="ps_z")
        first = True
        for c in range(2):
            wt = wgs[c]
            for k in range(KT):
                nc.tensor.matmul(zps[:], h2b[k][:, c:c + 1],
                                 wt[:, k * T:(k + 1) * T],
                                 start=first,
                                 stop=(c == 1 and k == KT - 1))
                first = False
        zpart = small.tile([1, T], F32, tag="zpart", name="zpart")
        nc.vector.tensor_copy(zpart[:], zps[:])
        zin = dram.tile([1, T], F32, name="zin")
        zout = dram.tile([1, T], F32, name="zout")
        # bounce DMAs on the scalar ring: the sync FIFO is full of u
        # chunks and would head-of-line-block these
        nc.scalar.dma_start(zin[:], zpart[:])
        nc.gpsimd.collective_compute(
            "AllReduce", mybir.AluOpType.add,
            replica_groups=[list(range(NCORES))],
            ins=[zin.opt()], outs=[zout.opt()])
        zrow = small.tile([1, T], BF16, tag="zrow", name="zrow")
        nc.gpsimd.dma_start(zrow[:], zout[:])  # SWDGE casts f32->bf16

        # ---------- logits = dcl @ av + z (broadcast over rows) ----------
        lgp = psum.tile([R, T], F32, tag="ps_lg", name="ps_lg")
        for k in range(KT):
            nc.tensor.matmul(lgp[:], dclT[k], av[k],
                             start=(k == 0), stop=False)
        nc.tensor.matmul(lgp[:], onesb[0:1, :R], zrow[:], start=False, stop=True)
        # matmul operands need base partition in {0, 32, 64}; pack the 50
        # L = exp(logits) rows (bf16) into 3 lanes at those partitions,
        # 17 rows each along the free dim. Bounce through DRAM to reshape
        # partitions->free in 3 DMAs.
        LPL = 17  # logits rows per lane
        lgb = small.tile([R, T], BF16, tag="lgb", name="lgb")
        nc.scalar.activation(lgb[:], lgp[:],
                             mybir.ActivationFunctionType.Exp)
        # bounce on the scalar ring: the sync ring is busy streaming u
        # chunks and would head-of-line-block these behind them
        ld = dram.tile([R, T], BF16, name="ldram")
        nc.scalar.dma_start(ld[:], lgb[:])
        fl = small.tile([65, LPL * T], BF16, tag="lgflat", name="lgflat")
        nc.scalar.dma_start(
            fl[0:33:32, :].rearrange("l (j t) -> l j t", j=LPL),
            ld[0:2 * LPL].rearrange("(l j) t -> l j t", l=2))
        nc.scalar.dma_start(fl[64:65, :(R - 2 * LPL) * T],
                            ld[2 * LPL:R].rearrange("(o j) t -> o (j t)", o=1))

        def lg_slice(r):
            lane, j = r // LPL, r % LPL
            return (fl[lane * 32:lane * 32 + 1, j * T:(j + 1) * T],
                    onesb[lane * 32:lane * 32 + 1, :SP])

        # ---------- main sampling loop ----------
        # u is (SP, R, T): each partition (sample) owns a contiguous
        # R*T*4 = 100KB DRAM run. Stream CH r's per chunk so every DMA
        # moves CH*2KB contiguous per partition (large packets), compute
        # a = -1/ln u in 3 chunk-wide in-place ACT passes, then per r:
        # PE-broadcast the L row into PSUM, multiply (+row-sum), then
        # one chunk-wide reciprocal and per-r normalize into bf16.
        for r0 in range(0, R, CH):
            if r0 in pre_ut:
                ut = pre_ut[r0]
            else:
                ut = upool.tile([SP, CW], F32, tag="u", name="u")
                nc.sync.dma_start(
                    ut[:].rearrange("p (c t) -> p c t", c=CH),
                    din["u"][:, r0:r0 + CH, :])
            # a = exp(-ln(-ln u)) = -1/ln(u), three chunk-wide ACT
            # passes (one table set), all independent of the logits.
            # Pass 3 lands in a separate bf16 tile so ut recycles at
            # ACT pace (not r-loop pace).
            nc.scalar.activation(ut[:], ut[:], mybir.ActivationFunctionType.Ln)
            nc.scalar.activation(ut[:], ut[:], mybir.ActivationFunctionType.Ln,
                                 scale=-1.0)
            at = apool.tile([SP, CW], BF16, tag="a", name="a")
            nc.scalar.activation(at[:], ut[:], mybir.ActivationFunctionType.Exp,
                                 scale=-1.0)
            ot = opool.tile([SP, CW], BF16, tag="o", name="o")
            ssc = spool.tile([SP, CH], F32, tag="ss", name="ss")
            rsc = spool.tile([SP, CH], F32, tag="rs", name="rs")
            qts = []
            for g in range(CH):
                seg = slice(g * T, (g + 1) * T)
                # broadcast L row r across partitions via a ones-matmul
                rhs, lhs_ones = lg_slice(r0 + g)
                bt = bppool.tile([SP, 512], F32, tag="bp", name="bp")
                nc.tensor.matmul(bt[:, :T], lhs_ones, rhs,
                                 start=True, stop=True)
                # q = a * L_bcast with fused row-sum
                # (tensor_tensor_reduce fails NEFF-side on this stack;
                # scalar_tensor_tensor with op0=bypass is HW-proven;
                # an ACT-engine PSUM->SBUF bcast copy before the stt
                # took the device down UNRECOVERABLE - do not retry).
                # q goes to a per-r tile, not an ot slice: in-place
                # chains on one chunk tile serialize all 5 r's.
                qt = qpool.tile([SP, T], BF16, tag="q", name="q")
                nc.vector.scalar_tensor_tensor(
                    qt[:], bt[:, :T], 0.0, at[:, seg],
                    op0=mybir.AluOpType.bypass, op1=mybir.AluOpType.mult,
                    accum_out=ssc[:, g:g + 1])
                qts.append(qt)
            nc.vector.reciprocal(rsc[:], ssc[:])
            for g in range(CH):
                seg = slice(g * T, (g + 1) * T)
                # (keep on DVE: a Pool-engine tensor_scalar_mul here
                # measured 3x WORSE end-to-end)
                nc.vector.tensor_scalar_mul(ot[:, seg], qts[g][:],
                                            rsc[:, g:g + 1])
            nc.gpsimd.dma_start(out[:, r0:r0 + CH, :],
                                ot[:].rearrange("p (c t) -> p c t", c=CH))


def _get_nc():
    if "nc" not in _CACHE:
        _CACHE["nc"] = _build()
    return _CACHE["nc"]


def prep_in_maps(inputs):
    import ml_dtypes
    f32 = np.float32
    bf16 = ml_dtypes.bfloat16
    state = np.asarray(inputs["state"], f32)[0]          # (500, 2)
    payoff = np.asarray(inputs["payoff"], f32)           # (500, 500)
    noise = np.asarray(inputs["feat_noise"], f32)[0]     # (500, 2)
    xT = np.concatenate([state, payoff, noise], axis=1).T.copy()  # (504, 500)
    gamma = np.asarray(inputs["bn_gamma"], f32)
    beta = np.asarray(inputs["bn_beta"], f32)
    adjT = (np.asarray(inputs["norm_adj"], f32) * gamma[:, None]).T
    dclT = np.asarray(inputs["def_cur_loc"], f32).T
    wr_full = np.asarray(inputs["actgen_w"], f32).reshape(T, H2, T)
    wr_full = wr_full.transpose(1, 0, 2)                 # (16, 500, 500)
    # per-core 2-channel shards, partition-contiguous:
    # wr_pack[g][c, p, k*T + t] = wr_full[2g + c, k*125 + p, t]
    wr_all = np.ascontiguousarray(
        wr_full.reshape(H2, KT, 125, T).transpose(0, 2, 1, 3)
    ).reshape(H2, 125, KT * T).astype(bf16)
    # mega-packed param planes (see _build)
    adjb = adjT.astype(bf16)    # (500, 500): k-tile rows k*125..
    avb = np.asarray(inputs["actgen_v"], f32).astype(bf16)
    dclb = dclT.astype(bf16)    # (500, 50)
    pbf = np.concatenate(
        [np.concatenate([adjb[k * 125:(k + 1) * 125] for k in range(KT)], axis=1),
         np.concatenate([avb[k * 125:(k + 1) * 125] for k in range(KT)], axis=1),
         np.concatenate([dclb[k * 125:(k + 1) * 125] for k in range(KT)], axis=1)],
        axis=1)                 # (125, 8*500 + 4*50)
    w1f = np.asarray(inputs["gc1_w"], f32)
    pxb = np.concatenate(
        [np.concatenate([xT[k * 126:(k + 1) * 126] for k in range(KT)], axis=1),
         np.concatenate([w1f[k * 126:(k + 1) * 126] for k in range(KT)], axis=1)],
        axis=1).astype(bf16)    # (126, 4*500 + 4*32)
    common = {
        "pbf": np.ascontiguousarray(pbf),
        "pxb": np.ascontiguousarray(pxb),
        "ident": np.eye(128, dtype=f32),
    }
    w2f = np.asarray(inputs["gc2_w"], f32)
    b2f = np.asarray(inputs["gc2_b"], f32).reshape(-1)
    b1f = np.asarray(inputs["gc1_b"], f32).reshape(-1)
    u = np.asarray(inputs["gumbel_u"], f32)              # (1000, 50, 500)
    in_maps = []
    for i in range(NCORES):
        m = dict(common)
        # permute gc2's output channels so this core's z-shard channels
        # (2i, 2i+1) sit at h2[:, 0:2]; channels only feed z, so
        # nothing else changes
        perm = [2 * i, 2 * i + 1] + [c for c in range(H2)
                                     if c not in (2 * i, 2 * i + 1)]
        m["w2"] = np.ascontiguousarray(w2f[:, perm])
        m["rows"] = np.concatenate(
            [b1f, b2f[perm], gamma, beta]).reshape(1, -1).astype(bf16)
        m["wr"] = np.ascontiguousarray(wr_all[2 * i:2 * i + 2])
        m["u"] = np.ascontiguousarray(u[i * SP:(i + 1) * SP])  # (125, 50, 500)
        in_maps.append(m)
    return in_maps


def run(inputs, trace=False):
    nc = _get_nc()
    in_maps = prep_in_maps(inputs)
    res = bass_utils.run_bass_kernel_spmd(
        nc, in_maps, core_ids=list(range(NCORES)), trace=trace)
    full = np.concatenate(
        [np.asarray(res.results[i]["out"]).astype(np.float32)
         for i in range(NCORES)], axis=0)                # (1000, 50, 500)
    return full, res


def kernel(**inputs):
    full, _ = run(inputs)
    return full


# revision 44
# speedup vs baseline: 2.7958x; 1.0593x over previous
"""Trainium2 Bass kernel for nn_Def_A2C_Sample_Generator.

Computation (see reference):
  x = concat(state, payoff, noise)            (500, 504)
  h1 = lrelu(bn(adj @ (x @ w1) + b1))         (500, 32)
  h2 = lrelu(bn(adj @ (h1 @ w2) + b2))        (500, 16)
  xf = h2.reshape(8000)
  logits = xf @ actgen_w + def_cur_loc @ actgen_v          (50, 500)
  out = softmax(logits[None] + gumbel(u), axis=-1)         (1000, 50, 500)

Sharding: data-parallel over the 1000 samples, 125 per core on 8
cores; actgen_w is additionally channel-sharded 2-of-16 per core with
a 2KB ncfw AllReduce joining the z partials (gc2's output channels
are host-permuted per core so each core's shard sits at h2[:, 0:2]).

THE GOVERNING CONSTRAINT (measured, not the doc numbers): per-core
sustained HBM bandwidth is only ~133 GB/s with all 8 cores streaming,
and exec time ~= total bytes / 133GB/s for every variant tried. Byte
budget per core: u 12.5MB f32 (MUST stay f32: a = -1/ln u amplifies
input error by 1/(1-u)) + out 6.25MB bf16 + wr shard 1MB + params
~1.7MB ~= 21.5MB. The AllReduce's ~65us latency is fully hidden:
the DVE tail only needs logits by ~T-50us.

Softmax factorization keeps all gumbel work independent of logits:
  exp(logits + g) with g = -ln(-ln u) equals L * a where
  L = exp(logits) (prologue row) and a = exp(-ln(-ln u)) = -1/ln u.
Main loop, 5-r chunks in the natural (sample, r, T) layout:
  a       : 3 chunk-wide ACT passes (Ln, Ln(-x) in-place f32, then
            Exp(-x) into a separate bf16 tile so the u tile recycles
            at ACT pace -- tying u recycling to the r-loop (which
            waits on logits) starves ACT for the whole wr window);
            one table set - see the act-table monkeypatch below
  L bcast : per-r PE ones-matmul, single bf16 plane into f32 PSUM
  q, S    : DVE scalar_tensor_tensor mult (bf16 out) + fused row-sum
  out     : one DVE reciprocal per chunk + per-r tensor_scalar mult
            into a bf16 chunk tile, one 625KB store per chunk
            (host upcasts to f32; total rel err ~5e-3, gate is 2e-2)

All matmul operands that tolerate bf16 are bf16 (PE mid-p-state is
2x slow and f32 costs 4 cyc/row on top). Params are mega-packed into
two plane loads (each serial HWDGE DMA costs ~1-2us issue+completion,
so ~25 small tile loads would serialize into ~45us).

DMA queues: params + u on the sync HWDGE ring (emission order = FIFO
drain order; never issue big DMAs from the scalar ring -- HWDGE
descriptor gen executes on the issuing engine and would eat ACT
time), wr shard + zrow cast-load + output stores on the gpsimd SWDGE
ring, tiny logits/z bounces on the scalar ring (the sync FIFO would
head-of-line-block them behind queued u chunks).

Logits rows are packed into 3 lanes at base partitions 0/32/64 (the
only legal matmul operand bases) via a DRAM bounce.

Known-bad variants (measured): Pool-engine tensor_scalar_mul for the
normalize = 3x WORSE end-to-end; ACT-engine PSUM->SBUF bcast copies
before the stt = device UNRECOVERABLE; 2-pass Ln+Reciprocal = table
swaps + no gain while ACT is u-paced.
"""
import sys

if "/opt/trn_rl_repo" not in sys.path:
    sys.path.insert(0, "/opt/trn_rl_repo")

import numpy as np

import concourse.bacc as bacc
import concourse.bass as bass
import concourse.mybir as mybir
import concourse.tile as tile
from concourse import bass_utils

# The act-table-load pass resolves Exp -> exp_and_others (id 0) and
# Ln -> natural_log (id 5), thrashing a ~2.7us table swap at every
# Ln<->Exp transition in the main loop. natural_log_exp_and_others
# (id 6) holds BOTH. Hide exp/ln from the other sets in the map the
# chooser reads (ids keep indexing the real act_info.json, so the
# loaded tables are unchanged) so every Exp and Ln lands on set 6 and
# one load suffices.
_orig_get_act_tables = bacc.get_activation_tables


def _patched_get_act_tables(arch):
    tabs = dict(_orig_get_act_tables(arch))
    both = {mybir.ActivationFunctionType.Exp, mybir.ActivationFunctionType.Ln}
    for name, fns in tabs.items():
        if name != "natural_log_exp_and_others" and (both & fns):
            tabs[name] = fns - both
    return tabs


bacc.get_activation_tables = _patched_get_act_tables

F32 = mybir.dt.float32
BF16 = mybir.dt.bfloat16
NCORES = 8
T = 500
R = 50
NS = 1000
SP = NS // NCORES  # 125 samples per core
H1, H2 = 32, 16
FIN = 504  # 2 + 500 + 2 input features
KT = 4  # K/M tiling of the 500 dim into 4x125
NEG_SLOPE = 0.2

_CACHE = {}


def _build():
    nc = bacc.Bacc("TRN2", target_bir_lowering=False, debug=False,
                   enable_asserts=False, num_devices=NCORES)

    # ---- I/O ----
    din = {}
    # mega-packed params: per-DMA fixed costs on the HWDGE ring are
    # ~1-2us each, so ~25 small tile loads serialize into ~45us. Two
    # packed planes load in ~8us instead.
    # pbf[p, :] = adjT k-tiles (4x500) | av k-tiles (4x500) | dclT (4x50)
    din["pbf"] = nc.dram_tensor("pbf", [125, 8 * T + 4 * R], BF16,
                                kind="ExternalInput")
    # pxb[p, :] = xT k-tiles (4x500) | w1 k-tiles (4x32), bf16
    din["pxb"] = nc.dram_tensor("pxb", [126, 4 * T + 4 * H1], BF16,
                                kind="ExternalInput")
    # rows[0, :] = b1 (32) | b2 (16) | grow (500) | brow (500)
    din["rows"] = nc.dram_tensor("rows", [1, H1 + H2 + 2 * T], BF16,
                                 kind="ExternalInput")
    din["w2"] = nc.dram_tensor("w2", [H1, H2], F32, kind="ExternalInput")
    # per-core actgen_w shard: 2 of 16 channels (the kernel is HBM-byte
    # bound at ~133GB/s/core, so the 8MB replicated stream IS the
    # bottleneck; each core computes a partial z from its 2 channels
    # and a 2KB ncfw AllReduce -- whose ~tens-of-us latency is fully
    # hidden, DVE only needs logits at ~110us -- completes it).
    # Host-side, gc2's output channels are permuted per core so the
    # owned channels sit at h2[:, 0:2]; channels only feed z.
    din["wr"] = nc.dram_tensor("wr", [2, 125, KT * T], BF16,
                               kind="ExternalInput")
    din["ident"] = nc.dram_tensor("ident", [128, 128], F32, kind="ExternalInput")
    din["u"] = nc.dram_tensor("u", [SP, R, T], F32, kind="ExternalInput")
    out = nc.dram_tensor("out", [SP, R, T], BF16, kind="ExternalOutput")

    with tile.TileContext(nc) as tc:
        _emit(nc, tc, din, out)
    nc.compile()
    return nc


def _emit(nc, tc, din, out):
    from contextlib import ExitStack

    ctx = ExitStack()
    with ctx:
        # ---------- pools ----------
        const = ctx.enter_context(tc.tile_pool(name="const", bufs=1))
        small = ctx.enter_context(tc.tile_pool(name="small", bufs=1))
        psum = ctx.enter_context(tc.tile_pool(name="psum", bufs=1, space="PSUM"))
        dram = ctx.enter_context(tc.tile_pool(name="dram", bufs=1, space="DRAM"))

        # ---------- pools for the main loop ----------
        CH = 5  # r's per chunk
        CW = CH * T
        # u tiles recycle at ACT pace (pass 3 writes `a` to a separate
        # bf16 tile): the u stream must NOT be gated on the r-loop,
        # which waits for logits, which waits for the wr stream --
        # tying those together starves ACT for the whole wr window.
        upool = ctx.enter_context(tc.tile_pool(name="upool", bufs=3))
        apool = ctx.enter_context(tc.tile_pool(name="apool", bufs=R // CH))
        opool = ctx.enter_context(tc.tile_pool(name="opool", bufs=3))
        qpool = ctx.enter_context(tc.tile_pool(name="qpool", bufs=6))
        spool = ctx.enter_context(tc.tile_pool(name="spool", bufs=4))
        bppool = ctx.enter_context(tc.tile_pool(name="bppool", bufs=5,
                                                space="PSUM"))

        # ---------- load params (HEAD of the sync FIFO: the HWDGE ring
        # drains in emission order, so the GCN/logits critical path must
        # come before the 12.5MB u stream) ----------
        onesb = const.tile([65, 128], BF16, tag="onesb", name="onesb")
        nc.vector.memset(onesb[:], 1.0)

        pbf = const.tile([125, 8 * T + 4 * R], BF16, tag="pbf", name="pbf")
        nc.sync.dma_start(pbf[:], din["pbf"][:])
        pxb = const.tile([126, 4 * T + 4 * H1], BF16, tag="pxb", name="pxb")
        nc.sync.dma_start(pxb[:], din["pxb"][:])
        rows = const.tile([1, H1 + H2 + 2 * T], BF16, tag="rows", name="rows")
        nc.sync.dma_start(rows[:], din["rows"][:])
        w2 = const.tile([H1, H2], F32, tag="w2", name="w2")
        nc.sync.dma_start(w2[:], din["w2"][:])
        ident = const.tile([128, 128], F32, tag="ident", name="ident")
        nc.sync.dma_start(ident[:], din["ident"][:])

        adjT = [pbf[:, k * T:(k + 1) * T] for k in range(KT)]
        av = [pbf[:, (KT + k) * T:(KT + k + 1) * T] for k in range(KT)]
        dclT = [pbf[:, 8 * T + k * R:8 * T + (k + 1) * R] for k in range(KT)]
        xT = [pxb[:, k * T:(k + 1) * T] for k in range(KT)]
        w1 = [pxb[:, 4 * T + k * H1:4 * T + (k + 1) * H1] for k in range(KT)]
        b1 = rows[0:1, 0:H1]
        b2 = rows[0:1, H1:H1 + H2]
        grow = rows[0:1, H1 + H2:H1 + H2 + T]
        brow = rows[0:1, H1 + H2 + T:H1 + H2 + 2 * T]

        # first two u chunks ahead of the weight stream so ACT starts
        # at ~13us (ACT consumes u at ~175GB/s = the whole DMA ceiling;
        # it will starve during the wr window regardless, but an early
        # start overlaps the GCN)
        pre_ut = {}
        for r0 in (0, CH):
            ut = upool.tile([SP, CW], F32, tag="u", name="u")
            nc.sync.dma_start(
                ut[:].rearrange("p (c t) -> p c t", c=CH),
                din["u"][:, r0:r0 + CH, :])
            pre_ut[r0] = ut

        # per-core wr shard (2 channels, 1MB) on the gpsimd ring
        wpool = ctx.enter_context(tc.tile_pool(name="wpool", bufs=2))
        wgs = []
        for g in range(2):
            wt = wpool.tile([125, KT * T], BF16, tag="wr_stream",
                            name="wr_stream")
            nc.gpsimd.dma_start(wt[:], din["wr"][g])
            wgs.append(wt)

        # ---------- GCN, transposed formulation ----------
        # bn is folded into the adjacency on the host (adjT ships
        # gamma[t]*adj[t,u] transposed), leaving rank-1 bias terms:
        #   bn(adj@xw+b)^T[c,t] = (xw^T adj1^T)[c,t] + b[c]*gamma[t]
        #                         + beta[t]
        # so each adj product is ONE [H,500] PSUM accumulation of 4
        # K-tiles plus two K=1 bias matmuls, and layer 2 consumes h1T
        # directly as its stationary operand (no transposes, no bn DVE
        # chain).
        def lrelu_from_psum(ps_ap, out_tile, width):
            tmp = small.tile([width, T], F32, tag=f"lr{width}", name=f"lr{width}")
            nc.vector.tensor_scalar_mul(tmp[:], ps_ap, NEG_SLOPE)
            nc.vector.tensor_tensor(out_tile[:], tmp[:], ps_ap,
                                    op=mybir.AluOpType.max)

        xw1 = [small.tile([125, H1], BF16, tag=f"xw1{m}", name=f"xw1{m}") for m in range(KT)]
        for m in range(KT):
            ps = psum.tile([125, H1], F32, tag="ps_small", name="ps_small")
            for k in range(KT):
                nc.tensor.matmul(ps[:], pxb[:, k * T + m * 125:k * T + (m + 1) * 125],
                                 w1[k], start=(k == 0), stop=(k == KT - 1))
            nc.vector.tensor_copy(xw1[m][:], ps[:])

        a1ps = psum.tile([H1, T], F32, tag="ps_small", name="ps_small")
        for k in range(KT):
            nc.tensor.matmul(a1ps[:], xw1[k][:], adjT[k],
                             start=(k == 0), stop=False)
        nc.tensor.matmul(a1ps[:], b1, grow, start=False, stop=False)
        nc.tensor.matmul(a1ps[:], onesb[0:1, :H1], brow, start=False,
                         stop=True)
        h1T = small.tile([H1, T], F32, tag="h1T", name="h1T")
        lrelu_from_psum(a1ps[:], h1T, H1)

        xw2 = [small.tile([125, H2], BF16, tag=f"xw2{m}", name=f"xw2{m}") for m in range(KT)]
        for m in range(KT):
            ps = psum.tile([125, H2], F32, tag="ps_small", name="ps_small")
            nc.tensor.matmul(ps[:], h1T[:, m * 125:(m + 1) * 125], w2[:],
                             start=True, stop=True)
            nc.vector.tensor_copy(xw2[m][:], ps[:])

        a2ps = psum.tile([H2, T], F32, tag="ps_small", name="ps_small")
        for k in range(KT):
            nc.tensor.matmul(a2ps[:], xw2[k][:], adjT[k],
                             start=(k == 0), stop=False)
        nc.tensor.matmul(a2ps[:], b2, grow, start=False, stop=False)
        nc.tensor.matmul(a2ps[:], onesb[0:1, :H2], brow, start=False,
                         stop=True)
        h2T = small.tile([H2, T], F32, tag="h2T", name="h2T")
        lrelu_from_psum(a2ps[:], h2T, H2)

        # h2 back to [t, c] tiles in bf16 for the z matmuls
        h2b = [small.tile([125, H2], BF16, tag=f"h2b{k}", name=f"h2b{k}")
               for k in range(KT)]
        for k in range(KT):
            pt = psum.tile([125, H2], F32, tag="ps_small", name="ps_small")
            nc.tensor.transpose(pt[:], h2T[:, k * 125:(k + 1) * 125],
                                ident[:H2, :H2])
            nc.vector.tensor_copy(h2b[k][:], pt[:])

        # ---------- z partial (2 owned channels) + AllReduce ----------
        zps = psum.tile([1, T], F32, tag="ps_z", name="ps_z")
        first = True
        for c in range(2):
            wt = wgs[c]
            for k in range(KT):
                nc.tensor.matmul(zps[:], h2b[k][:, c:c + 1],
                                 wt[:, k * T:(k + 1) * T],
                                 start=first,
                                 stop=(c == 1 and k == KT - 1))
                first = False
        zpart = small.tile([1, T], F32, tag="zpart", name="zpart")
        nc.vector.tensor_copy(zpart[:], zps[:])
        zin = dram.tile([1, T], F32, name="zin")
        zout = dram.tile([1, T], F32, name="zout")
        # bounce DMAs on the scalar ring: the sync FIFO is full of u
        # chunks and would head-of-line-block these
        nc.scalar.dma_start(zin[:], zpart[:])
        nc.gpsimd.collective_compute(
            "AllReduce", mybir.AluOpType.add,
            replica_groups=[list(range(NCORES))],
            ins=[zin.opt()], outs=[zout.opt()])
        zrow = small.tile([1, T], BF16, tag="zrow", name="zrow")
        nc.gpsimd.dma_start(zrow[:], zout[:])  # SWDGE casts f32->bf16

        # ---------- logits = dcl @ av + z (broadcast over rows) ----------
        lgp = psum.tile([R, T], F32, tag="ps_lg", name="ps_lg")
        for k in range(KT):
            nc.tensor.matmul(lgp[:], dclT[k], av[k],
                             start=(k == 0), stop=False)
        nc.tensor.matmul(lgp[:], onesb[0:1, :R], zrow[:], start=False, stop=True)
        # matmul operands need base partition in {0, 32, 64}; pack the 50
        # L = exp(logits) rows (bf16) into 3 lanes at those partitions,
        # 17 rows each along the free dim. Bounce through DRAM to reshape
        # partitions->free in 3 DMAs.
        LPL = 17  # logits rows per lane
        lgb = small.tile([R, T], BF16, tag="lgb", name="lgb")
        nc.scalar.activation(lgb[:], lgp[:],
                             mybir.ActivationFunctionType.Exp)
        # bounce on the scalar ring: the sync ring is busy streaming u
        # chunks and would head-of-line-block these behind them
        ld = dram.tile([R, T], BF16, name="ldram")
        nc.scalar.dma_start(ld[:], lgb[:])
        fl = small.tile([65, LPL * T], BF16, tag="lgflat", name="lgflat")
        nc.scalar.dma_start(
            fl[0:33:32, :].rearrange("l (j t) -> l j t", j=LPL),
            ld[0:2 * LPL].rearrange("(l j) t -> l j t", l=2))
        nc.scalar.dma_start(fl[64:65, :(R - 2 * LPL) * T],
                            ld[2 * LPL:R].rearrange("(o j) t -> o (j t)", o=1))

        def lg_slice(r):
            lane, j = r // LPL, r % LPL
            return (fl[lane * 32:lane * 32 + 1, j * T:(j + 1) * T],
                    onesb[lane * 32:lane * 32 + 1, :SP])

        # ---------- main sampling loop ----------
        # u is (SP, R, T): each partition (sample) owns a contiguous
        # R*T*4 = 100KB DRAM run. Stream CH r's per chunk so every DMA
        # moves CH*2KB contiguous per partition (large packets), compute
        # a = -1/ln u in 3 chunk-wide in-place ACT passes, then per r:
        # PE-broadcast the L row into PSUM, multiply (+row-sum), then
        # one chunk-wide reciprocal and per-r normalize into bf16.
        for r0 in range(0, R, CH):
            if r0 in pre_ut:
                ut = pre_ut[r0]
            else:
                ut = upool.tile([SP, CW], F32, tag="u", name="u")
                nc.sync.dma_start(
                    ut[:].rearrange("p (c t) -> p c t", c=CH),
                    din["u"][:, r0:r0 + CH, :])
            # a = exp(-ln(-ln u)) = -1/ln(u), three chunk-wide ACT
            # passes (one table set), all independent of the logits.
            # Pass 3 lands in a separate bf16 tile so ut recycles at
            # ACT pace (not r-loop pace).
            nc.scalar.activation(ut[:], ut[:], mybir.ActivationFunctionType.Ln)
            nc.scalar.activation(ut[:], ut[:], mybir.ActivationFunctionType.Ln,
                                 scale=-1.0)
            at = apool.tile([SP, CW], BF16, tag="a", name="a")
            nc.scalar.activation(at[:], ut[:], mybir.ActivationFunctionType.Exp,
                                 scale=-1.0)
            ot = opool.tile([SP, CW], BF16, tag="o", name="o")
            ssc = spool.tile([SP, CH], F32, tag="ss", name="ss")
            rsc = spool.tile([SP, CH], F32, tag="rs", name="rs")
            qts = []
            for g in range(CH):
                seg = slice(g * T, (g + 1) * T)
                # broadcast L row r across partitions via a ones-matmul
                rhs, lhs_ones = lg_slice(r0 + g)
                bt = bppool.tile([SP, 512], F32, tag="bp", name="bp")
                nc.tensor.matmul(bt[:, :T], lhs_ones, rhs,
                                 start=True, stop=True)
                # q = a * L_bcast with fused row-sum
                # (tensor_tensor_reduce fails NEFF-side on this stack;
                # scalar_tensor_tensor with op0=bypass is HW-proven;
                # an ACT-engine PSUM->SBUF bcast copy before the stt
                # took the device down UNRECOVERABLE - do not retry).
                # q goes to a per-r tile, not an ot slice: in-place
                # chains on one chunk tile serialize all 5 r's.
                qt = qpool.tile([SP, T], BF16, tag="q", name="q")
                nc.vector.scalar_tensor_tensor(
                    qt[:], bt[:, :T], 0.0, at[:, seg],
                    op0=mybir.AluOpType.bypass, op1=mybir.AluOpType.mult,
                    accum_out=ssc[:, g:g + 1])
                qts.append(qt)
            nc.vector.reciprocal(rsc[:], ssc[:])
            for g in range(CH):
                seg = slice(g * T, (g + 1) * T)
                # (keep on DVE: a Pool-engine tensor_scalar_mul here
                # measured 3x WORSE end-to-end)
                nc.vector.tensor_scalar_mul(ot[:, seg], qts[g][:],
                                            rsc[:, g:g + 1])
            nc.gpsimd.dma_start(out[:, r0:r0 + CH, :],
                                ot[:].rearrange("p (c t) -> p c t", c=CH))


def _get_nc():
    if "nc" not in _CACHE:
        _CACHE["nc"] = _build()
    return _CACHE["nc"]


def prep_in_maps(inputs):
    import ml_dtypes
    f32 = np.float32
    bf16 = ml_dtypes.bfloat16
    state = np.asarray(inputs["state"], f32)[0]          # (500, 2)
    payoff = np.asarray(inputs["payoff"], f32)           # (500, 500)
    noise = np.asarray(inputs["feat_noise"], f32)[0]     # (500, 2)
    xT = np.concatenate([state, payoff, noise], axis=1).T.copy()  # (504, 500)
    gamma = np.asarray(inputs["bn_gamma"], f32)
    beta = np.asarray(inputs["bn_beta"], f32)
    adjT = (np.asarray(inputs["norm_adj"], f32) * gamma[:, None]).T
    dclT = np.asarray(inputs["def_cur_loc"], f32).T
    wr_full = np.asarray(inputs["actgen_w"], f32).reshape(T, H2, T)
    wr_full = wr_full.transpose(1, 0, 2)                 # (16, 500, 500)
    # per-core 2-channel shards, partition-contiguous:
    # wr_pack[g][c, p, k*T + t] = wr_full[2g + c, k*125 + p, t]
    wr_all = np.ascontiguousarray(
        wr_full.reshape(H2, KT, 125, T).transpose(0, 2, 1, 3)
    ).reshape(H2, 125, KT * T).astype(bf16)
    # mega-packed param planes (see _build)
    adjb = adjT.astype(bf16)    # (500, 500): k-tile rows k*125..
    avb = np.asarray(inputs["actgen_v"], f32).astype(bf16)
    dclb = dclT.astype(bf16)    # (500, 50)
    pbf = np.concatenate(
        [np.concatenate([adjb[k * 125:(k + 1) * 125] for k in range(KT)], axis=1),
         np.concatenate([avb[k * 125:(k + 1) * 125] for k in range(KT)], axis=1),
         np.concatenate([dclb[k * 125:(k + 1) * 125] for k in range(KT)], axis=1)],
        axis=1)                 # (125, 8*500 + 4*50)
    w1f = np.asarray(inputs["gc1_w"], f32)
    pxb = np.concatenate(
        [np.concatenate([xT[k * 126:(k + 1) * 126] for k in range(KT)], axis=1),
         np.concatenate([w1f[k * 126:(k + 1) * 126] for k in range(KT)], axis=1)],
        axis=1).astype(bf16)    # (126, 4*500 + 4*32)
    common = {
        "pbf": np.ascontiguousarray(pbf),
        "pxb": np.ascontiguousarray(pxb),
        "ident": np.eye(128, dtype=f32),
    }
    w2f = np.asarray(inputs["gc2_w"], f32)
    b2f = np.asarray(inputs["gc2_b"], f32).reshape(-1)
    b1f = np.asarray(inputs["gc1_b"], f32).reshape(-1)
    u = np.asarray(inputs["gumbel_u"], f32)              # (1000, 50, 500)
    in_maps = []
    for i in range(NCORES):
        m = dict(common)
        # permute gc2's output channels so this core's z-shard channels
        # (2i, 2i+1) sit at h2[:, 0:2]; channels only feed z, so
        # nothing else changes
        perm = [2 * i, 2 * i + 1] + [c for c in range(H2)
                                     if c not in (2 * i, 2 * i + 1)]
        m["w2"] = np.ascontiguousarray(w2f[:, perm])
        m["rows"] = np.concatenate(
            [b1f, b2f[perm], gamma, beta]).reshape(1, -1).astype(bf16)
        m["wr"] = np.ascontiguousarray(wr_all[2 * i:2 * i + 2])
        m["u"] = np.ascontiguousarray(u[i * SP:(i + 1) * SP])  # (125, 50, 500)
        in_maps.append(m)
    return in_maps


def run(inputs, trace=False):
    nc = _get_nc()
    in_maps = prep_in_maps(inputs)
    res = bass_utils.run_bass_kernel_spmd(
        nc, in_maps, core_ids=list(range(NCORES)), trace=trace)
    full = np.concatenate(
        [np.asarray(res.results[i]["out"]).astype(np.float32)
         for i in range(NCORES)], axis=0)                # (1000, 50, 500)
    return full, res


def kernel(**inputs):
    full, _ = run(inputs)
    return full
